# revision 1
# baseline (speedup 1.0000x reference)
"""Trainium2 Bass kernel: disparity regression via top-2 over the last axis.

pred[b, n] = sum_k topi_k * softmax(topv_k)  with K=2 over cost[b, n, :192].

Strategy (vs. the max8+max_index baseline, which was DVE-bound at ~2 full
passes over the data): pack (quantized value, index) into a single fp32 per
element so ONE DVE max8 pass yields both top-2 values and indices:

  ACT:   u = Identity(x*2^21 + (2^31+2^23))   -- fp32 RNE at ulp=256 forces
         u = (2^31+2^23) + 256*q,  q = round(8192*x)   (low 8 bits clear)
  index combine  p = u - (2^31+2^23-2^21) + ((255-d) - 2^21), load-balanced
  per super-tile across three engines (none can absorb it alone):
    'D' DVE:     one custom-DVE affine_then_add (1 elem/cyc)
    'T' TensorE: identity fp32 matmul of u into PSUM, then bf16 k=1
                 broadcast matmuls accumulate -2^31, -2^23, +(255-d)
                 (all bf16-exact); max8 then reads PSUM
    'G' GPSIMD:  ACT cancels C2 (per-partition bias), Pool adds the index
                 row in place
   ->  p = 256*q + (255-d): exact integer in fp32, monotone in (quantized
       value, -d); equal quantized values pick the lowest index (lax.top_k).
  DVE:   max8 per row on p (the only full-data DVE selection pass)
  epilogue: inv via a tie-free fp32 round-to-256 trick; d = 255-inv;
         s = sigmoid((q256_2-q256_1)/2^21); pred = (255+invn1) + (d2-d1)*s

Value quantization step is 2^-13; elements with x < -4 fall out of the
ulp=256 region and merely get coarser quantization - they can never be
top-2. Rel err ~2e-3 from sub-quantum v2/v3 ties, well under the 2e-2 gate.
"""
import numpy as np

import concourse.bacc as bacc
import concourse.tile as tile
import concourse.mybir as mybir
from concourse.bass_utils import run_bass_kernel_spmd

N_CORES = 8
B, N, D = 4, 131072, 192
ROWS = B * N                       # 524288
ROWS_PER_CORE = ROWS // N_CORES    # 65536
P = 128                            # SBUF partitions
G = 32                             # rows per partition per super-tile
TILE_ROWS = P * G                  # 4096
N_TILES = ROWS_PER_CORE // TILE_ROWS  # 16
COLS = G * D                       # 6144
CH_COLS = 1536                     # PSUM chunk: 8 rows = 3 banks
BK = 512                           # fp32 per PSUM bank
NQ = 2                             # DMA chunks per super-tile

F32 = mybir.dt.float32
BF16 = mybir.dt.bfloat16
I32 = mybir.dt.int32
AF = mybir.ActivationFunctionType
OP = mybir.AluOpType

SCALE = float(2 ** 21)             # 8192 * 256
BIAS = float(2 ** 31 + 2 ** 23)
INV_OFF = float(2 ** 21)
C2 = INV_OFF - BIAS
C31 = -float(2 ** 31)
C23 = -float(2 ** 23)

# per-super-tile combine routing: D=DVE, T=TensorE, G=GPSIMD
ROUTE = "DTDGDTDDDTDGDTDD"


def build(loop_iters: int = 1):
    nc = bacc.Bacc(
        "TRN2", target_bir_lowering=False, debug=False, num_devices=N_CORES
    )
    x = nc.dram_tensor("cost", [ROWS_PER_CORE, D], F32, kind="ExternalInput").ap()
    y = nc.dram_tensor("pred", [ROWS_PER_CORE], F32, kind="ExternalOutput").ap()

    x_t = x.rearrange("(t p g) d -> t p (g d)", p=P, g=G)
    y_t = y.rearrange("(t p g) -> t p g", p=P, g=G)

    ident_d = nc.inline_tensor(np.eye(P, dtype=np.float32), name="ident")

    def consts(tc, cp, tp):
        biast = cp.tile([P, 1], F32)
        nc.gpsimd.memset(biast[:], BIAS)
        c2t = cp.tile([P, 1], F32)
        nc.gpsimd.memset(c2t[:], C2)
        dgrid = tp.tile([P, COLS], mybir.dt.int16)
        nc.gpsimd.iota(dgrid[:], pattern=[[0, G], [1, D]], base=0,
                       channel_multiplier=0)
        invt = cp.tile([P, COLS], F32)
        nc.vector.tensor_scalar(invt[:], dgrid[:], -1.0, 255.0 - INV_OFF,
                                OP.mult, OP.add)
        ident = cp.tile([P, P], F32)
        nc.sync.dma_start(ident[:], ident_d.ap())
        ones_b = cp.tile([1, P], BF16)
        nc.gpsimd.memset(ones_b[:], 1.0)
        c31row = cp.tile([1, BK], BF16)
        nc.gpsimd.memset(c31row[:], C31)
        c23row = cp.tile([1, BK], BF16)
        nc.gpsimd.memset(c23row[:], C23)
        drow = tp.tile([1, 768], I32)
        nc.gpsimd.iota(drow[:], pattern=[[0, 4], [1, D]], base=0,
                       channel_multiplier=0)
        invrow = cp.tile([1, 768], BF16)       # 255-d (bf16-exact, TE path)
        nc.vector.tensor_scalar(invrow[:], drow[:], -1.0, 255.0,
                                OP.mult, OP.add)
        return biast, c2t, invt, ident, ones_b, c31row, c23row, invrow

    def body(tc, cn, xp, up, pp, qp, vp, ep):
        biast, c2t, invt, ident, ones_b, c31row, c23row, invrow = cn
        for t in range(N_TILES):
            route = ROUTE[t]
            xt = xp.tile([P, COLS], F32)
            for q in range(NQ):
                c0, c1 = q * (COLS // NQ), (q + 1) * (COLS // NQ)
                nc.sync.dma_start(xt[:, c0:c1], x_t[t][:, c0:c1])

            ut = up.tile([P, COLS], F32)
            nc.scalar.activation(ut[:], xt[:], AF.Identity,
                                 bias=biast[:], scale=SCALE)

            v8 = vp.tile([P, G * 8], F32)
            if route == "T":
                for ch in range(COLS // CH_COLS):
                    ps = qp.tile([P, CH_COLS], F32)
                    for b in range(CH_COLS // BK):
                        col = ch * CH_COLS + b * BK
                        dst = ps[:, b * BK:(b + 1) * BK]
                        nc.tensor.matmul(dst, ident[:], ut[:, col:col + BK],
                                         start=True, stop=False)
                        off = col % D
                        for j, row in enumerate((c31row[:], c23row[:],
                                                 invrow[:, off:off + BK])):
                            nc.tensor.matmul(dst, ones_b[:], row,
                                             start=False, stop=(j == 2))
                    for g in range(8):
                        r = ch * 8 + g
                        nc.vector.max(v8[:, r * 8:(r + 1) * 8],
                                      ps[:, g * D:(g + 1) * D])
            else:
                pk = pp.tile([P, COLS], F32)
                if route == "G":
                    nc.scalar.activation(pk[:], ut[:], AF.Identity,
                                         bias=c2t[:])
                    nc.gpsimd.tensor_add(pk[:], pk[:], invt[:])
                else:
                    nc.vector.affine_then_add(pk[:], ut[:], invt[:],
                                              scale=1.0, bias=C2)
                for g in range(G):
                    nc.vector.max(v8[:, g * 8:(g + 1) * 8],
                                  pk[:, g * D:(g + 1) * D])

            v8v = v8[:].rearrange("p (g k) -> p g k", k=8)
            pair = v8v[:, :, 0:2]                     # [P, G, 2]

            # inv = p mod 256 via a tie-free fp32 round-trip: pm = p-128
            # (inv-128 in [-64,127], never a .5 tie), q256 = RNE-to-256(pm)
            # = 256*q, invn = -inv = (q256-128) - pm.  All exact in fp32.
            pm = ep.tile([P, 2 * G], F32)
            pmv = pm[:].rearrange("p (g k) -> p g k", k=2)
            nc.vector.tensor_single_scalar(pmv, pair, -128.0, OP.add)
            q256 = ep.tile([P, 2 * G], F32)
            nc.vector.tensor_scalar(q256[:], pm[:], BIAS, BIAS,
                                    OP.add, OP.subtract)
            invn = ep.tile([P, 2 * G], F32)           # -inv per slot
            nc.vector.scalar_tensor_tensor(invn[:], q256[:], -128.0, pm[:],
                                           OP.add, OP.subtract)
            q_v = q256[:].rearrange("p (g k) -> p g k", k=2)
            inv_v = invn[:].rearrange("p (g k) -> p g k", k=2)

            sm = ep.tile([P, G], F32)                 # 256*(q2-q1) <= 0
            nc.gpsimd.tensor_sub(sm[:], q_v[:, :, 1], q_v[:, :, 0])
            invd = ep.tile([P, G], F32)               # inv1-inv2 = d2-d1
            nc.gpsimd.tensor_sub(invd[:], inv_v[:, :, 1], inv_v[:, :, 0])
            s = ep.tile([P, G], F32)
            nc.scalar.activation(s[:], sm[:], AF.Sigmoid, scale=1.0 / SCALE)
            w = ep.tile([P, G], F32)
            nc.gpsimd.tensor_mul(w[:], invd[:], s[:])
            pt = ep.tile([P, G], F32)                 # (invn1+255) + w
            nc.vector.scalar_tensor_tensor(pt[:], inv_v[:, :, 0], 255.0,
                                           w[:], OP.add, OP.add)

            nc.sync.dma_start(y_t[t], pt[:])

    with tile.TileContext(nc) as tc:
        with (
            tc.tile_pool(name="cp", bufs=1) as cp,
            tc.tile_pool(name="xp", bufs=2) as xp,
            tc.tile_pool(name="up", bufs=2) as up,
            tc.tile_pool(name="pp", bufs=2) as pp,
            tc.tile_pool(name="qp", bufs=2, space="PSUM") as qp,
            tc.tile_pool(name="vp", bufs=4) as vp,
            tc.tile_pool(name="ep", bufs=4) as ep,
        ):
            with tc.tile_pool(name="tp", bufs=1) as tp:
                cn = consts(tc, cp, tp)
            if loop_iters == 1:
                body(tc, cn, xp, up, pp, qp, vp, ep)
            else:
                with tc.For_i(0, loop_iters, 1):
                    body(tc, cn, xp, up, pp, qp, vp, ep)

    nc.compile()
    return nc


_NC_CACHE = {}


def _get_nc(loop_iters: int = 1):
    if loop_iters not in _NC_CACHE:
        _NC_CACHE[loop_iters] = build(loop_iters)
    return _NC_CACHE[loop_iters]


def run(cost: np.ndarray, loop_iters: int = 1) -> np.ndarray:
    nc = _get_nc(loop_iters)
    flat = np.ascontiguousarray(cost.reshape(ROWS, D))
    in_maps = [
        {"cost": flat[c * ROWS_PER_CORE:(c + 1) * ROWS_PER_CORE]}
        for c in range(N_CORES)
    ]
    res = run_bass_kernel_spmd(nc, in_maps, core_ids=list(range(N_CORES)))
    out = np.concatenate(
        [res.results[c]["pred"] for c in range(N_CORES)]
    )
    return out.reshape(B, N).astype(np.float32, copy=False)


def kernel(cost: np.ndarray) -> np.ndarray:
    return run(cost, loop_iters=1)

